# revision 2
# baseline (speedup 1.0000x reference)
"""Single-head unscaled attention (B=8, T=2048, D=1024, NODES=1024) on 8 trn2 cores.

Sharding: data-parallel over batch - core b computes batch element b end-to-end.
Weights are replicated to every core.

v3 = v2 + the V projection overlapped with the first two attention tiles:
S_0/S_1 and their softmax are computed between V blocks, so when V finishes
the PV pipeline starts immediately (no phase transition stall).

Per-core pipeline (all matmuls fp16 in / fp32 PSUM accumulate):
  X^T  = PE-transpose(cast16(X))                     [d, t]
  K^T  = Wk^T X^T,  Q^T = Wq^T X^T  (lhsT=W tile)    [n, t]
  V    = X Wv       (lhsT=X^T tile)                  [t, n]
  per q-tile (128 rows):
    S    = Q^T.T K^T   -> PSUM 4x[128, 512]
    softmax: block max (DVE) -> row max -> exp+row-sum fused on ACT -> P fp16
    P^T  via PE transpose (16x [128,128] into group PSUM tiles, batched copies)
    O    = P^T.T V     -> PSUM [128, 1024];  O *= 1/rowsum;  DMA out
"""

from contextlib import ExitStack

import numpy as np

import concourse.bass as bass
import concourse.mybir as mybir
import concourse.tile as tile
from concourse import bacc
from concourse.bass import ts
from concourse.masks import make_identity

P = 128
T = 2048
D = 1024
NO = 1024
B = 8
TT = T // P   # 16 tiles of 128 along t
DT = D // P   # 8 tiles along d
NT = NO // P  # 8 tiles along nodes
KB = 4        # 512-wide key blocks per row of S
QB = 4        # 512-wide t blocks in projections

F16 = mybir.dt.float16
F32 = mybir.dt.float32
AX = mybir.AxisListType
EXP = mybir.ActivationFunctionType.Exp
MAX = mybir.AluOpType.max
ADD = mybir.AluOpType.add


def _attention_body(tc, out, x, wq, wk, wv):
    nc = tc.nc
    x3 = x.rearrange("(t p) d -> t p d", p=P)
    o3 = out.rearrange("(t p) n -> t p n", p=P)

    with ExitStack() as ctx:
        const = ctx.enter_context(tc.tile_pool(name="const", bufs=1))
        persist = ctx.enter_context(tc.tile_pool(name="persist", bufs=1))
        xtpool = ctx.enter_context(tc.tile_pool(name="xtp", bufs=1))
        w16pool = ctx.enter_context(tc.tile_pool(name="w16p", bufs=2))
        wstage = ctx.enter_context(tc.tile_pool(name="wstage", bufs=3))

        ident = const.tile([P, P], F16, tag="ident")
        make_identity(nc, ident)

        qt = persist.tile([P, NT, T], F16, tag="qt")    # Q^T [n_in, n_out, t]
        kt = persist.tile([P, NT, T], F16, tag="kt")    # K^T
        v = persist.tile([P, TT, NO], F16, tag="v")     # V   [t_in, t_out, n]
        xt = xtpool.tile([P, DT, T], F16, tag="xt")     # X^T [d_in, d_out, t]

        def load_w(wap):
            w16 = w16pool.tile([P, DT, NO], F16, tag="w16")
            w3 = wap.rearrange("(do p) n -> do p n", p=P)
            for do in range(DT):
                ws = wstage.tile([P, NO], F32, tag="ws")
                nc.sync.dma_start(ws, w3[do])
                # DVE cast: keeps the ACT semaphore domain exclusive to
                # x-tile casts so PE transposes never wait on w casts
                nc.vector.tensor_copy(w16[:, do], ws)
            return w16

        # ---------------- phase A1: loads, X^T, K and Q projections --------
        with tc.tile_pool(name="xstage", bufs=3) as xstage, tc.tile_pool(
            name="xh", bufs=2
        ) as xhpool, tc.tile_pool(
            name="ppsum", bufs=6, space="PSUM"
        ) as ppsum, tc.tile_pool(name="tpsA", bufs=2, space="PSUM") as tpsA:

            def load_x_tile(t_):
                xs = xstage.tile([P, D], F32, tag="xs")
                nc.sync.dma_start(xs, x3[t_])
                xh = xhpool.tile([P, D], F16, tag="xh")
                nc.scalar.copy(xh, xs)  # ACT cast (DVE does proj copies)
                tp = tpsA.tile([P, DT, P], F16, tag="tpa")
                for do in range(DT):
                    nc.tensor.transpose(tp[:, do], xh[:, ts(do, P)], ident)
                nc.vector.tensor_copy(xt[:, :, ts(t_, P)], tp)

            def proj_block(w16, dst, no, qb):
                ps = ppsum.tile([P, 512], F32, tag="pp")
                for do in range(DT):
                    nc.tensor.matmul(
                        ps,
                        w16[:, do, ts(no, P)],
                        xt[:, do, ts(qb, 512)],
                        start=(do == 0),
                        stop=(do == DT - 1),
                    )
                nc.vector.tensor_copy(dst[:, no, ts(qb, 512)], ps)

            # DMA issue is serialized on the SP queue in program order at
            # ~316 GB/s, so order loads by when compute needs them
            for t_ in range(0, 4):
                load_x_tile(t_)
            wk16 = load_w(wk)
            # K qb0 do-major with 6 open PSUM chains: each do-step needs only
            # wk stage `do`, so matmuls start as stages land instead of
            # waiting for the whole Wk DMA
            ps6 = [
                ppsum.tile([P, 512], F32, tag="pp", name=f"ps6_{i}")
                for i in range(6)
            ]
            for do in range(DT):
                for no in range(6):
                    nc.tensor.matmul(
                        ps6[no],
                        wk16[:, do, ts(no, P)],
                        xt[:, do, ts(0, 512)],
                        start=(do == 0),
                        stop=(do == DT - 1),
                    )
            for no in range(6):
                nc.vector.tensor_copy(kt[:, no, ts(0, 512)], ps6[no])
            for no in range(6, NT):
                proj_block(wk16, kt, no, 0)
            for t_ in range(4, 8):
                load_x_tile(t_)
            for no in range(NT):
                proj_block(wk16, kt, no, 1)
            for t_ in range(8, 12):
                load_x_tile(t_)
            for no in range(NT):
                proj_block(wk16, kt, no, 2)
            for t_ in range(12, 16):
                load_x_tile(t_)
            for no in range(NT):
                proj_block(wk16, kt, no, 3)
            wq16 = load_w(wq)
            for qb in range(QB):
                for no in range(NT):
                    proj_block(wq16, qt, no, qb)

        # ------------- phase A2 + B: V overlapped with attention ----------
        with tc.tile_pool(name="spsum", bufs=KB, space="PSUM") as spsum, tc.tile_pool(
            name="tps", bufs=2, space="PSUM"
        ) as tps, tc.tile_pool(name="soft", bufs=2) as soft, tc.tile_pool(
            name="p16p", bufs=2
        ) as p16pool, tc.tile_pool(name="ptp", bufs=2) as ptpool, tc.tile_pool(
            name="outp", bufs=2
        ) as outp:

            def emit_s_blocks(q_, kbs, sblks, bmax, at_kb2=None):
                for kb in kbs:
                    sb = spsum.tile([P, 512], F32, tag="sb")
                    for no in range(NT):
                        nc.tensor.matmul(
                            sb,
                            qt[:, no, ts(q_, P)],
                            kt[:, no, ts(kb, 512)],
                            start=(no == 0),
                            stop=(no == NT - 1),
                        )
                    nc.vector.tensor_reduce(
                        bmax[:, kb : kb + 1], sb, axis=AX.X, op=MAX
                    )
                    sblks.append(sb)
                    if kb == KB - 2 and at_kb2 is not None:
                        at_kb2()

            def emit_softmax(q_, sblks, bmax, drain_pt=False, after_negmax=None):
                rmax = soft.tile([P, 1], F32, tag="rmax")
                nc.vector.tensor_reduce(rmax, bmax, axis=AX.X, op=MAX)
                negmax = soft.tile([P, 1], F32, tag="negmax")
                nc.vector.tensor_scalar_mul(negmax, rmax, -1.0)
                if after_negmax is not None:
                    after_negmax()  # emit_pv of the previous tile
                p16 = p16pool.tile([P, T], F16, tag="p16")
                bsum = soft.tile([P, KB], F32, tag="bsum")
                ptt = (
                    ptpool.tile([P, TT, P], F16, tag="ptt", name="ptt_drain")
                    if drain_pt
                    else None
                )
                for kb in range(KB):
                    nc.scalar.activation(
                        p16[:, ts(kb, 512)],
                        sblks[kb],
                        EXP,
                        bias=negmax,
                        scale=1.0,
                        accum_out=bsum[:, kb : kb + 1],
                    )
                    if drain_pt:
                        # drain: transpose each exp block as it lands so the
                        # final PV isn't serialized behind the whole softmax
                        g = kb * 4
                        tp = tps.tile([P, 4, P], F16, tag="tp")
                        for j in range(4):
                            nc.tensor.transpose(
                                tp[:, j], p16[:, ts(g + j, P)], ident
                            )
                        nc.vector.tensor_copy(ptt[:, g : g + 4], tp)
                rsum = soft.tile([P, 1], F32, tag="rsum")
                nc.vector.tensor_reduce(rsum, bsum, axis=AX.X, op=ADD)
                inv = soft.tile([P, 1], F32, tag="inv")
                nc.vector.reciprocal(inv, rsum)
                return p16, inv, ptt

            def emit_pt(p16):
                # P^T via PE transposes into group PSUM tiles, batched copies
                ptt = ptpool.tile([P, TT, P], F16, tag="ptt")
                for g in range(0, TT, 4):
                    tp = tps.tile([P, 4, P], F16, tag="tp")
                    for j in range(4):
                        nc.tensor.transpose(tp[:, j], p16[:, ts(g + j, P)], ident)
                    nc.vector.tensor_copy(ptt[:, g : g + 4], tp)
                return ptt

            # --- A2: V projection with S_0 / S_1 + softmax interleaved ---
            with tc.tile_pool(name="ppsumB", bufs=2, space="PSUM") as ppsumB:

                def v_blocks(ts_range):
                    for t_ in ts_range:
                        for nb in range(2):
                            ps = ppsumB.tile([P, 512], F32, tag="pv")
                            for do in range(DT):
                                nc.tensor.matmul(
                                    ps,
                                    xt[:, do, ts(t_, P)],
                                    wv16[:, do, ts(nb, 512)],
                                    start=(do == 0),
                                    stop=(do == DT - 1),
                                )
                            nc.vector.tensor_copy(v[:, t_, ts(nb, 512)], ps)

                wv16 = load_w(wv)
                v_blocks(range(0, 8))
                sblks0, bmax0 = [], soft.tile([P, KB], F32, tag="bmax")
                emit_s_blocks(0, range(KB), sblks0, bmax0)
                p16_0, inv_0, _ = emit_softmax(0, sblks0, bmax0)
                v_blocks(range(8, 12))
                sblks1, bmax1 = [], soft.tile([P, KB], F32, tag="bmax")
                emit_s_blocks(1, range(KB), sblks1, bmax1)
                p16_1, inv_1, _ = emit_softmax(1, sblks1, bmax1)
                v_blocks(range(12, 16))

            # --- B: steady-state attention loop from q=2 ---
            with tc.tile_pool(name="opsum", bufs=1, space="PSUM") as opsum:

                def emit_pv(q_, ptt, inv):
                    o = opsum.tile([P, 2, 512], F32, tag="o")
                    ob = outp.tile([P, NO], F32, tag="ob")
                    for nb in range(2):
                        for k_ in range(TT):
                            nc.tensor.matmul(
                                o[:, nb],
                                ptt[:, k_, :],
                                v[:, k_, ts(nb, 512)],
                                start=(k_ == 0),
                                stop=(k_ == TT - 1),
                            )
                        nc.vector.tensor_scalar_mul(
                            ob[:, ts(nb, 512)], o[:, nb], inv
                        )
                        # split out-DMA so the last tile's first half
                        # overlaps its second PV chain
                        nc.sync.dma_start(
                            o3[q_][:, ts(nb, 512)], ob[:, ts(nb, 512)]
                        )

                # tile 0's PV goes first; its P^T transposes cover the
                # PSUM WAR latency from ppsumB closing
                ptt0 = emit_pt(p16_0)
                emit_pv(0, ptt0, inv_0)

                prev = (1, p16_1, inv_1)
                ptt_last = None
                for q_ in range(2, TT):
                    sblks = []
                    bmax = soft.tile([P, KB], F32, tag="bmax")

                    def at_kb2():
                        # previous tile's P^T transposes go on the tensor
                        # queue before this tile's last S block: the DVE
                        # reaches the copies while S block 3 runs, so PV
                        # starts with its inputs ready
                        nonlocal prev
                        prev = (prev[0], emit_pt(prev[1]), prev[2])

                    emit_s_blocks(q_, range(KB), sblks, bmax, at_kb2)
                    last = q_ == TT - 1
                    pv_prev = prev

                    p16, inv, ptt_l = emit_softmax(
                        q_,
                        sblks,
                        bmax,
                        drain_pt=last,
                        after_negmax=lambda: emit_pv(*pv_prev),
                    )
                    if last:
                        ptt_last = ptt_l
                    prev = (q_, p16, inv)
                emit_pv(prev[0], ptt_last, prev[2])


_CACHED_NC = {}


# revision 8
# speedup vs baseline: 1.0376x; 1.0376x over previous
"""Single-head unscaled attention (B=8, T=2048, D=1024, NODES=1024) on 8 trn2 cores.

Sharding: data-parallel over batch - core b computes batch element b end-to-end.
Weights are replicated to every core.

v3 = v2 + the V projection overlapped with the first two attention tiles:
S_0/S_1 and their softmax are computed between V blocks, so when V finishes
the PV pipeline starts immediately (no phase transition stall).

Per-core pipeline (all matmuls fp16 in / fp32 PSUM accumulate):
  X^T  = PE-transpose(cast16(X))                     [d, t]
  K^T  = Wk^T X^T,  Q^T = Wq^T X^T  (lhsT=W tile)    [n, t]
  V    = X Wv       (lhsT=X^T tile)                  [t, n]
  per q-tile (128 rows):
    S    = Q^T.T K^T   -> PSUM 4x[128, 512]
    softmax: block max (DVE) -> row max -> exp+row-sum fused on ACT -> P fp16
    P^T  via PE transpose (16x [128,128] into group PSUM tiles, batched copies)
    O    = P^T.T V     -> PSUM [128, 1024];  O *= 1/rowsum;  DMA out
"""

from contextlib import ExitStack

import numpy as np

import concourse.bass as bass
import concourse.mybir as mybir
import concourse.tile as tile
from concourse import bacc
from concourse.bass import ts
from concourse.masks import make_identity

P = 128
T = 2048
D = 1024
NO = 1024
B = 8
TT = T // P   # 16 tiles of 128 along t
DT = D // P   # 8 tiles along d
NT = NO // P  # 8 tiles along nodes
KB = 4        # 512-wide key blocks per row of S
QB = 4        # 512-wide t blocks in projections

F16 = mybir.dt.float16
F32 = mybir.dt.float32
AX = mybir.AxisListType
EXP = mybir.ActivationFunctionType.Exp
MAX = mybir.AluOpType.max
ADD = mybir.AluOpType.add


def _attention_body(tc, out, x, wq, wk, wv):
    nc = tc.nc
    x3 = x.rearrange("(t p) d -> t p d", p=P)
    o3 = out.rearrange("(t p) n -> t p n", p=P)

    with ExitStack() as ctx:
        const = ctx.enter_context(tc.tile_pool(name="const", bufs=1))
        persist = ctx.enter_context(tc.tile_pool(name="persist", bufs=1))
        xtpool = ctx.enter_context(tc.tile_pool(name="xtp", bufs=1))
        w16pool = ctx.enter_context(tc.tile_pool(name="w16p", bufs=2))
        wstage = ctx.enter_context(tc.tile_pool(name="wstage", bufs=3))

        ident = const.tile([P, P], F16, tag="ident")
        make_identity(nc, ident)

        qt = persist.tile([P, NT, T], F16, tag="qt")    # Q^T [n_in, n_out, t]
        kt = persist.tile([P, NT, T], F16, tag="kt")    # K^T
        v = persist.tile([P, TT, NO], F16, tag="v")     # V   [t_in, t_out, n]
        xt = xtpool.tile([P, DT, T], F16, tag="xt")     # X^T [d_in, d_out, t]

        def load_w(wap):
            w16 = w16pool.tile([P, DT, NO], F16, tag="w16")
            w3 = wap.rearrange("(do p) n -> do p n", p=P)
            for do in range(DT):
                ws = wstage.tile([P, NO], F32, tag="ws")
                nc.sync.dma_start(ws, w3[do])
                # DVE cast: keeps the ACT semaphore domain exclusive to
                # x-tile casts so PE transposes never wait on w casts
                nc.vector.tensor_copy(w16[:, do], ws)
            return w16

        # ---------------- phase A1: loads, X^T, K and Q projections --------
        with tc.tile_pool(name="xstage", bufs=4) as xstage, tc.tile_pool(
            name="xh", bufs=4
        ) as xhpool, tc.tile_pool(
            name="ppsum", bufs=6, space="PSUM"
        ) as ppsum, tc.tile_pool(name="tpsA", bufs=2, space="PSUM") as tpsA:

            def load_x_tile(t_):
                xs = xstage.tile([P, D], F32, tag="xs")
                nc.sync.dma_start(xs, x3[t_])
                xh = xhpool.tile([P, D], F16, tag="xh")
                nc.scalar.copy(xh, xs)  # ACT cast (DVE does proj copies)
                for h in range(2):
                    tp = tpsA.tile([P, 4, P], F16, tag="tpa", name=f"tpa{h}")
                    for j in range(4):
                        do = 4 * h + j
                        nc.tensor.transpose(tp[:, j], xh[:, ts(do, P)], ident)
                    nc.vector.tensor_copy(
                        xt[:, 4 * h : 4 * h + 4, ts(t_, P)], tp
                    )

            def proj_block(w16, dst, no, qb):
                ps = ppsum.tile([P, 512], F32, tag="pp")
                for do in range(DT):
                    nc.tensor.matmul(
                        ps,
                        w16[:, do, ts(no, P)],
                        xt[:, do, ts(qb, 512)],
                        start=(do == 0),
                        stop=(do == DT - 1),
                    )
                nc.vector.tensor_copy(dst[:, no, ts(qb, 512)], ps)

            # DMA issue is serialized on the SP queue in program order at
            # ~316 GB/s, so order loads by when compute needs them
            for t_ in range(0, 4):
                load_x_tile(t_)
            wk16 = load_w(wk)
            # K qb0 do-major with 6 open PSUM chains: each do-step needs only
            # wk stage `do`, so matmuls start as stages land instead of
            # waiting for the whole Wk DMA
            ps6 = [
                ppsum.tile([P, 512], F32, tag="pp", name=f"ps6_{i}")
                for i in range(6)
            ]
            for do in range(DT):
                for no in range(6):
                    nc.tensor.matmul(
                        ps6[no],
                        wk16[:, do, ts(no, P)],
                        xt[:, do, ts(0, 512)],
                        start=(do == 0),
                        stop=(do == DT - 1),
                    )
            for no in range(6):
                nc.vector.tensor_copy(kt[:, no, ts(0, 512)], ps6[no])
            for no in range(6, NT):
                proj_block(wk16, kt, no, 0)
            for t_ in range(4, 8):
                load_x_tile(t_)
            for no in range(NT):
                proj_block(wk16, kt, no, 1)
            for t_ in range(8, 12):
                load_x_tile(t_)
            for no in range(NT):
                proj_block(wk16, kt, no, 2)
            for t_ in range(12, 16):
                load_x_tile(t_)
            for no in range(NT):
                proj_block(wk16, kt, no, 3)
            wq16 = load_w(wq)
            for qb in range(QB):
                for no in range(NT):
                    proj_block(wq16, qt, no, qb)

        # ------------- phase A2 + B: V overlapped with attention ----------
        with tc.tile_pool(name="spsum", bufs=KB, space="PSUM") as spsum, tc.tile_pool(
            name="tps", bufs=2, space="PSUM"
        ) as tps, tc.tile_pool(name="soft", bufs=2) as soft, tc.tile_pool(
            name="p16p", bufs=2
        ) as p16pool, tc.tile_pool(name="ptp", bufs=2) as ptpool, tc.tile_pool(
            name="outp", bufs=2
        ) as outp:

            def emit_s_blocks(q_, kbs, sblks, bmax, at_kb2=None):
                for kb in kbs:
                    sb = spsum.tile([P, 512], F32, tag="sb")
                    for no in range(NT):
                        nc.tensor.matmul(
                            sb,
                            qt[:, no, ts(q_, P)],
                            kt[:, no, ts(kb, 512)],
                            start=(no == 0),
                            stop=(no == NT - 1),
                        )
                    nc.vector.tensor_reduce(
                        bmax[:, kb : kb + 1], sb, axis=AX.X, op=MAX
                    )
                    sblks.append(sb)
                    if kb == KB - 2 and at_kb2 is not None:
                        at_kb2()

            def emit_softmax(q_, sblks, bmax, drain_pt=False, after_negmax=None):
                rmax = soft.tile([P, 1], F32, tag="rmax")
                nc.vector.tensor_reduce(rmax, bmax, axis=AX.X, op=MAX)
                negmax = soft.tile([P, 1], F32, tag="negmax")
                nc.vector.tensor_scalar_mul(negmax, rmax, -1.0)
                if after_negmax is not None:
                    after_negmax()  # emit_pv of the previous tile
                p16 = p16pool.tile([P, T], F16, tag="p16")
                bsum = soft.tile([P, KB], F32, tag="bsum")
                ptt = (
                    ptpool.tile([P, TT, P], F16, tag="ptt", name="ptt_drain")
                    if drain_pt
                    else None
                )
                for kb in range(KB):
                    nc.scalar.activation(
                        p16[:, ts(kb, 512)],
                        sblks[kb],
                        EXP,
                        bias=negmax,
                        scale=1.0,
                        accum_out=bsum[:, kb : kb + 1],
                    )
                    if drain_pt:
                        # drain: transpose each exp block as it lands so the
                        # final PV isn't serialized behind the whole softmax
                        g = kb * 4
                        tp = tps.tile([P, 4, P], F16, tag="tp")
                        for j in range(4):
                            nc.tensor.transpose(
                                tp[:, j], p16[:, ts(g + j, P)], ident
                            )
                        nc.vector.tensor_copy(ptt[:, g : g + 4], tp)
                rsum = soft.tile([P, 1], F32, tag="rsum")
                nc.vector.tensor_reduce(rsum, bsum, axis=AX.X, op=ADD)
                inv = soft.tile([P, 1], F32, tag="inv")
                nc.vector.reciprocal(inv, rsum)
                return p16, inv, ptt

            def emit_pt(p16):
                # P^T via PE transposes into group PSUM tiles, batched copies
                ptt = ptpool.tile([P, TT, P], F16, tag="ptt")
                for g in range(0, TT, 4):
                    tp = tps.tile([P, 4, P], F16, tag="tp")
                    for j in range(4):
                        nc.tensor.transpose(tp[:, j], p16[:, ts(g + j, P)], ident)
                    nc.vector.tensor_copy(ptt[:, g : g + 4], tp)
                return ptt

            # --- A2: V projection with S_0 / S_1 + softmax interleaved ---
            with tc.tile_pool(name="ppsumB", bufs=2, space="PSUM") as ppsumB:

                def v_blocks(ts_range):
                    for t_ in ts_range:
                        for nb in range(2):
                            ps = ppsumB.tile([P, 512], F32, tag="pv")
                            for do in range(DT):
                                nc.tensor.matmul(
                                    ps,
                                    xt[:, do, ts(t_, P)],
                                    wv16[:, do, ts(nb, 512)],
                                    start=(do == 0),
                                    stop=(do == DT - 1),
                                )
                            nc.vector.tensor_copy(v[:, t_, ts(nb, 512)], ps)

                wv16 = load_w(wv)
                v_blocks(range(0, 8))
                sblks0, bmax0 = [], soft.tile([P, KB], F32, tag="bmax")
                emit_s_blocks(0, range(KB), sblks0, bmax0)
                p16_0, inv_0, _ = emit_softmax(0, sblks0, bmax0)
                v_blocks(range(8, 12))
                sblks1, bmax1 = [], soft.tile([P, KB], F32, tag="bmax")
                emit_s_blocks(1, range(KB), sblks1, bmax1)
                p16_1, inv_1, _ = emit_softmax(1, sblks1, bmax1)
                v_blocks(range(12, 16))

            # --- B: steady-state attention loop from q=2 ---
            with tc.tile_pool(name="opsum", bufs=1, space="PSUM") as opsum:

                def emit_pv(q_, ptt, inv):
                    o = opsum.tile([P, 2, 512], F32, tag="o")
                    ob = outp.tile([P, NO], F32, tag="ob")
                    for nb in range(2):
                        for k_ in range(TT):
                            nc.tensor.matmul(
                                o[:, nb],
                                ptt[:, k_, :],
                                v[:, k_, ts(nb, 512)],
                                start=(k_ == 0),
                                stop=(k_ == TT - 1),
                            )
                        nc.vector.tensor_scalar_mul(
                            ob[:, ts(nb, 512)], o[:, nb], inv
                        )
                        # split out-DMA so the last tile's first half
                        # overlaps its second PV chain
                        nc.sync.dma_start(
                            o3[q_][:, ts(nb, 512)], ob[:, ts(nb, 512)]
                        )

                # tile 0's PV goes first; its P^T transposes cover the
                # PSUM WAR latency from ppsumB closing
                ptt0 = emit_pt(p16_0)
                emit_pv(0, ptt0, inv_0)

                prev = (1, p16_1, inv_1)
                ptt_last = None
                for q_ in range(2, TT):
                    sblks = []
                    bmax = soft.tile([P, KB], F32, tag="bmax")

                    def at_kb2():
                        # previous tile's P^T transposes go on the tensor
                        # queue before this tile's last S block: the DVE
                        # reaches the copies while S block 3 runs, so PV
                        # starts with its inputs ready
                        nonlocal prev
                        prev = (prev[0], emit_pt(prev[1]), prev[2])

                    emit_s_blocks(q_, range(KB), sblks, bmax, at_kb2)
                    last = q_ == TT - 1
                    pv_prev = prev

                    p16, inv, ptt_l = emit_softmax(
                        q_,
                        sblks,
                        bmax,
                        drain_pt=last,
                        after_negmax=lambda: emit_pv(*pv_prev),
                    )
                    if last:
                        ptt_last = ptt_l
                    prev = (q_, p16, inv)
                emit_pv(prev[0], ptt_last, prev[2])


_CACHED_NC = {}
